# revision 41
# baseline (speedup 1.0000x reference)
"""Bass/Trainium2 kernel for nn_ExaoneMoEAttention (sliding-window GQA attention).

Strategy (8 NeuronCores, tensor-parallel over heads):
  - core c owns q heads 4c..4c+3 and kv head c (w_qkv column shard [4096, 768]),
    plus w_o rows 512c..512c+512 ([512, 4096]); hidden replicated.
  - Phase A (QKV proj): per 128-row t-tile, hidT tiles are the stationary
    operand and w_qkv columns the moving operand; RMSNorm stats via ACT
    Square+accum_out; normalized q/k tiles are PE-transposed to [d, t] strips
    (norm weight and softmax scale folded into the evacuation); RoPE via host
    cos/sin tables. The first two t-tiles are emitted w-chunk-major so the PE
    starts as soon as hid0 + w-chunk0 land and rides the weight stream.
  - Phase B: scoresT[s, q] tiles; 256-wide q chunks prune the causal/window
    block-sparsity at 128-key granularity (<=10 key tiles per chunk). The two
    heads of a GQA pair share kT/vnat and ride one matmul stream (512 free
    cols via a [d, 2, 256] AP). Softmax without max-subtraction; masking is
    multiplicative on the exp'd tiles (DVE) using two triangle patterns -
    every diagonal/window-edge mask reduces to [zeros|tri_lo] or
    [tri_hi|zeros] slices. Sums via ones-matmul; per-pair normalization by
    PE-broadcast reciprocal, deferred into the next pair's score stream.
    PSUM pools are shared across phases (same tags) so the phase boundary
    carries no pool-release stall; o_proj PSUM is evacuated on the Scalar
    engine to keep DVE off the critical chain.
  - o_proj per 256-row slab -> bf16 ReduceScatter(add) over 8 cores per slab
    (collectives are latency-floor bound, ~16-40us each, and serialize on the
    CC cores, so slabs RS as soon as their partial lands; each slab's o_proj
    drain is deferred into the next slab's first score tiles to keep the PE
    dense, except the last three slabs whose RS timing sets the tail). The
    final slab reduce-scatters as two 128-row halves so only ~one
    latency-floor op hangs past the last matmul. Host concatenates the
    row-shards.
"""

import ml_dtypes
import numpy as np

import concourse.mybir as mybir
import concourse.tile as tile
from concourse import bacc
from concourse.bass_utils import run_bass_kernel_spmd
from concourse.masks import make_identity

F32 = mybir.dt.float32
F32R = mybir.dt.float32r
BF16 = mybir.dt.bfloat16
AF = mybir.ActivationFunctionType

N_CORES = 8
T = 2048
HID = 4096
H = 32
HKV = 8
D = 128
EPS = 1e-5
THETA = 1e6
WINDOW = 1024

HL = H // N_CORES          # 4 local q heads
NT = T // 128              # 16 t/s tiles
KO = HID // 128            # 32 k-subtiles in projection
QC = 256                   # q chunk in attention phase
N_QC = T // QC             # 8
ECH = 512                  # o_proj e-chunk
N_ECH = HID // ECH         # 8


def _build():
    nc = bacc.Bacc(num_devices=N_CORES)

    # hidT4[ki, tt, ko, j] = hidden[tt*128+j, ko*128+ki]
    hidT = nc.declare_dram_parameter("hidT", [128, NT, KO, 128], BF16, isOutput=False)
    wq = nc.declare_dram_parameter("wq", [128, KO, (HL + 2) * D], BF16, isOutput=False)
    wo = nc.declare_dram_parameter("wo", [128, HL, HID], BF16, isOutput=False)
    cs2 = nc.declare_dram_parameter("cs2", [128, T], F32, isOutput=False)
    sn2s = nc.declare_dram_parameter("sn2s", [128, T], F32, isOutput=False)
    masks = nc.declare_dram_parameter("masks", [128, 2, 256], BF16, isOutput=False)
    qwv = nc.declare_dram_parameter("qwv", [D, 1], F32, isOutput=False)
    kwv = nc.declare_dram_parameter("kwv", [D, 1], F32, isOutput=False)
    onc_d = nc.declare_dram_parameter("onc", [128, 1], BF16, isOutput=False)
    onr_d = nc.declare_dram_parameter("onr", [1, 128], F32R, isOutput=False)
    out_p = nc.declare_dram_parameter("out", [N_QC, QC // N_CORES, HID], BF16, isOutput=True)

    with tile.TileContext(nc) as tc:
        with (
            tc.tile_pool(name="persistA", bufs=1) as pA,
            # PSUM pools span both phases (shared tags) — closing/reopening
            # psum pools at the phase boundary would stall Phase B's first
            # tiles on Phase A's full drain via released-zone deps.
            # ps512: A pq + B scores/o_proj/bcast (4x2KB); psB: A pq2 +
            # B av (3x2KB); psC: A transposes + B sums (1x2KB). 16KB total.
            tc.tile_pool(name="ps512", bufs=4, space="PSUM") as ps512_p,
            tc.tile_pool(name="psB", bufs=3, space="PSUM") as psB_p,
            tc.tile_pool(name="psC", bufs=1, space="PSUM") as psC_p,
        ):
            kT = pA.tile([128, T], F32R)                     # rope'd k, [d, s]
            qT = pA.tile([128, HL, T], F32R)                 # rope'd q, [d, h, t]
            vnat = pA.tile([128, NT, D], BF16)               # v in [s, d] tiles
            onc = pA.tile([128, 1], BF16)
            onr = pA.tile([1, 128], F32R)
            ident = pA.tile([128, 128], BF16)
            # Phase-B persistent tiles live here so their DMAs can be issued
            # mid-Phase-A (off the critical startup window)
            wo_sb = pA.tile([128, HL, HID], BF16)
            mask_sb = pA.tile([128, 2, 256], BF16)  # [zeros|tri_lo], [tri_hi|zeros]
            make_identity(nc, ident[:])
            nc.sync.dma_start(out=onc[:], in_=onc_d[:])
            nc.sync.dma_start(out=onr[:], in_=onr_d[:])

            # ---------------- Phase A: QKV projection + norm + rope ----------
            with (
                tc.tile_pool(name="wpool", bufs=1) as wpool,
                tc.tile_pool(name="hidp", bufs=4) as hidp,
                tc.tile_pool(name="cspool", bufs=4) as cspool,
                tc.tile_pool(name="tmpA", bufs=6) as tmpA,
                tc.tile_pool(name="stA", bufs=6) as stA,
                tc.tile_pool(name="miscA", bufs=1) as miscA,
            ):
                # DMA issue order shapes the startup ramp: the first matmuls
                # need only hid tile 0 + the first w chunk, so those go first
                # and the weight stream is split into 8 chunks with early hid
                # tiles interleaved (rope tables ride the gpsimd queue).
                prefetched = []

                def fetch_tt(tt):
                    tsl = slice(tt * 128, (tt + 1) * 128)
                    hid_t = hidp.tile([128, KO, 128], BF16, tag="hid")
                    nc.sync.dma_start(out=hid_t[:], in_=hidT[:, tt])
                    cs_t = cspool.tile([128, 128], F32, tag="cs")
                    sn_t = cspool.tile([128, 128], F32, tag="sn")
                    nc.gpsimd.dma_start(out=cs_t[:], in_=cs2[:, tsl])
                    nc.gpsimd.dma_start(out=sn_t[:], in_=sn2s[:, tsl])
                    prefetched.append((hid_t, cs_t, sn_t))

                NW = 8
                KW = KO // NW
                fetch_tt(0)
                w_grp = []
                for g in range(NW):
                    wt = wpool.tile([128, KW, (HL + 2) * D], BF16, name=f"w{g}")
                    nc.sync.dma_start(out=wt[:], in_=wq[:, g * KW:(g + 1) * KW, :])
                    w_grp.append(wt)
                    if g == 0:
                        fetch_tt(1)
                    elif g == 2:
                        fetch_tt(2)
                qw_sb = miscA.tile([D, 1], F32)
                kw_sb = miscA.tile([D, 1], F32)
                eps_sb = miscA.tile([128, 1], F32)
                nc.sync.dma_start(out=qw_sb[:], in_=qwv[:])
                nc.sync.dma_start(out=kw_sb[:], in_=kwv[:])
                nc.vector.memset(eps_sb[:], EPS)

                pending_post = []

                def flush_post(keep=0):
                    while len(pending_post) > keep:
                        pending_post.pop(0)()

                def make_post(tt, pq, pq2, tsl, cs_t, sn_t):
                    def _post():
                        # v copy first so pq2 drains early; m=0 is the k head
                        # (also pq2) so its chain leads each stage
                        nc.vector.tensor_copy(vnat[:, tt, :], pq2[:, D:2 * D])
                        # stage-parallel across the 5 normed heads so the
                        # ACT/DVE chains pipeline instead of serializing
                        srcs = [pq2[:, 0:D]] + [pq[:, m * D:(m + 1) * D] for m in range(HL)]
                        var, sd, rstd, ev, tp, qd, qsw = [], [], [], [], [], [], []
                        for m in range(HL + 1):
                            sqd = tmpA.tile([128, D], F32, tag="sqd", name="sqd")
                            var.append(stA.tile([128, 1], F32, tag="var", name="var"))
                            nc.scalar.activation(sqd[:], srcs[m], AF.Square, accum_out=var[m][:])
                        for m in range(HL + 1):
                            sd.append(stA.tile([128, 1], F32, tag="sd", name="sd"))
                            nc.scalar.activation(sd[m][:], var[m][:], AF.Sqrt, scale=1.0 / D, bias=eps_sb[:])
                        for m in range(HL + 1):
                            rstd.append(stA.tile([128, 1], F32, tag="rstd", name="rstd"))
                            nc.vector.reciprocal(rstd[m][:], sd[m][:])
                        for m in range(HL + 1):
                            ev.append(tmpA.tile([128, D], BF16, tag="ev", name="ev"))
                            nc.scalar.activation(ev[m][:], srcs[m], AF.Copy, scale=rstd[m][:])
                        for m in range(HL + 1):
                            tp.append(psC_p.tile([128, 128], BF16, tag="ts", name="tp"))
                            nc.tensor.transpose(tp[m][:], ev[m][:], ident[:])
                        for m in range(HL + 1):
                            qd.append(tmpA.tile([128, D], F32, tag="qd", name="qd"))
                            nc.scalar.activation(
                                qd[m][:], tp[m][:], AF.Copy,
                                scale=(kw_sb[:] if m == 0 else qw_sb[:]),
                            )
                        for m in range(HL + 1):
                            qsw.append(tmpA.tile([128, D], F32, tag="qsw", name="qsw"))
                            nc.vector.tensor_copy(qsw[m][0:64, :], qd[m][64:128, :])
                            nc.vector.tensor_copy(qsw[m][64:128, :], qd[m][0:64, :])
                        for m in range(HL + 1):
                            nc.vector.tensor_mul(qd[m][:], qd[m][:], cs_t[:])
                            nc.vector.tensor_mul(qsw[m][:], qsw[m][:], sn_t[:])
                            dst = kT[:, tsl] if m == 0 else qT[:, m - 1, tsl]
                            nc.vector.tensor_add(dst, qd[m][:], qsw[m][:])
                    return _post

                def emit_tile_mms(tt, pq, pq2, hid_t, g_range):
                    for g in g_range:
                        for k in range(KW):
                            ko = g * KW + k
                            lhsT = hid_t[:, ko, :]
                            nc.tensor.matmul(
                                pq[:], lhsT, w_grp[g][:, k, 0:4 * D],
                                start=(ko == 0), stop=(ko == KO - 1),
                            )
                            nc.tensor.matmul(
                                pq2[:], lhsT, w_grp[g][:, k, 4 * D:6 * D],
                                start=(ko == 0), stop=(ko == KO - 1),
                            )

                # ramp: tiles 0-2 emitted w-chunk-major (PE starts on chunk 0;
                # 3 tiles x 1.28us per w chunk outpaces the ~3.1us chunk
                # arrival so the PE saturates through the weight stream)
                RAMP = 3
                ramp = []
                for tt in range(RAMP):
                    pq = ps512_p.tile([128, 4 * D], F32, tag="sc", name=f"pqr{tt}")
                    pq2 = psB_p.tile([128, 2 * D], F32, tag="av", name=f"pq2r{tt}")
                    ramp.append((pq, pq2))
                for g in range(NW):
                    for tt in range(RAMP):
                        emit_tile_mms(tt, ramp[tt][0], ramp[tt][1], prefetched[tt][0], [g])
                for tt in range(RAMP):
                    hid_t, cs_t, sn_t = prefetched.pop(0)
                    tsl = slice(tt * 128, (tt + 1) * 128)
                    pending_post.append(make_post(tt, ramp[tt][0], ramp[tt][1], tsl, cs_t, sn_t))

                next_fetch = 3  # tiles 0-2 are always fetched pre-loop
                for tt in range(RAMP, NT):
                    tsl = slice(tt * 128, (tt + 1) * 128)
                    while next_fetch < min(tt + 4, NT):
                        fetch_tt(next_fetch)
                        next_fetch += 1
                    hid_t, cs_t, sn_t = prefetched.pop(0)
                    if tt == RAMP + 2:
                        # issued only after the w stream has drained — these
                        # share DMA engines with the sync queue and would
                        # dilute the startup weight ramp otherwise
                        nc.gpsimd.dma_start(out=mask_sb[:], in_=masks[:])
                    elif tt == RAMP + 4:
                        nc.gpsimd.dma_start(out=wo_sb[:], in_=wo[:])

                    # qkv[t, c] for this t-tile: [128, 512] + [128, 256] psums
                    pq = ps512_p.tile([128, 4 * D], F32, tag="sc", name="pq")
                    pq2 = psB_p.tile([128, 2 * D], F32, tag="av", name="pq2")
                    emit_tile_mms(tt, pq, pq2, hid_t, range(NW))
                    flush_post(keep=1)
                    pending_post.append(make_post(tt, pq, pq2, tsl, cs_t, sn_t))
                flush_post()

            # ---------------- Phase B: attention + o_proj + reduce-scatter ---
            with (
                tc.tile_pool(name="persistB", bufs=1) as pB,
                tc.tile_pool(name="exp", bufs=8) as exp_p,
                tc.tile_pool(name="stB", bufs=2) as stB,
                tc.tile_pool(name="ostg", bufs=8) as ostg_p,
                tc.tile_pool(name="dramB", bufs=1, space="DRAM") as dramB,
            ):
                attnT = pB.tile([128, HL, T], BF16)

                partial = [
                    dramB.tile([QC, HID], BF16, name=f"partial{qc}") for qc in range(N_QC)
                ]
                rs_out = [
                    dramB.tile([QC // N_CORES, HID], BF16, name=f"rsout{qc}")
                    for qc in range(N_QC)
                ]
                rs_half = [
                    dramB.tile([QC // (2 * N_CORES), HID], BF16, name=f"rshalf{i}")
                    for i in range(2)
                ]

                pending_norm = [None]

                def flush_norm():
                    if pending_norm[0] is not None:
                        pending_norm[0]()
                        pending_norm[0] = None

                flush_norm2 = flush_norm

                oproj_q = []

                def emit_oproj_group():
                    if oproj_q:
                        oproj_q.pop(0)()

                def queue_oproj(qc):
                    def emit_rs(qc, half=None):
                        if half is None:
                            nc.gpsimd.collective_compute(
                                "ReduceScatter",
                                mybir.AluOpType.add,
                                replica_groups=[list(range(N_CORES))],
                                ins=[partial[qc][:]],
                                outs=[rs_out[qc][:]],
                            )
                            # same gpsimd queue as the RS, so this wait can't
                            # poison other engines
                            nc.gpsimd.dma_start(out=out_p[qc], in_=rs_out[qc][:])
                        else:
                            # final slab: two 128-row (1MB) halves so the RS
                            # past the last matmul is one latency-floor op
                            nc.gpsimd.collective_compute(
                                "ReduceScatter",
                                mybir.AluOpType.add,
                                replica_groups=[list(range(N_CORES))],
                                ins=[partial[qc][half * 128:(half + 1) * 128, :]],
                                outs=[rs_half[half][:]],
                            )
                            ho = QC // (2 * N_CORES)
                            nc.gpsimd.dma_start(
                                out=out_p[qc, half * ho:(half + 1) * ho, :],
                                in_=rs_half[half][:],
                            )

                    def make_group(trow, ec, rs_after):
                        def _g():
                            pso = ps512_p.tile([128, ECH], F32, tag="sc", name="pso")
                            for h in range(HL):
                                nc.tensor.matmul(
                                    pso[:],
                                    attnT[:, h, trow * 128:(trow + 1) * 128],
                                    wo_sb[:, h, ec * ECH:(ec + 1) * ECH],
                                    start=(h == 0),
                                    stop=(h == HL - 1),
                                )
                            ost = ostg_p.tile([128, ECH], BF16, tag="ost", name="ost")
                            # alternate evacuation engine to balance ACT/DVE
                            if ec % 2 == 0:
                                nc.scalar.activation(ost[:], pso[:], AF.Copy)
                            else:
                                nc.vector.tensor_copy(ost[:], pso[:])
                            nc.sync.dma_start(
                                out=partial[qc][(trow % 2) * 128:(trow % 2 + 1) * 128,
                                                ec * ECH:(ec + 1) * ECH],
                                in_=ost[:],
                            )
                            if rs_after is not None:
                                emit_rs(qc, None if rs_after < 0 else rs_after)
                        return _g

                    rows = [qc * 2 + tt for tt in range(QC // 128)]
                    last_slab = qc == N_QC - 1
                    i = 0
                    for qi, trow in enumerate(rows):
                        for ec in range(N_ECH):
                            i += 1
                            rs_after = None
                            if last_slab and i % N_ECH == 0:
                                rs_after = qi
                            elif i == len(rows) * N_ECH:
                                rs_after = -1
                            oproj_q.append(make_group(trow, ec, rs_after))

                # Per 256-wide q chunk, key tiles si run from the window edge
                # to the diagonal. Masking reduces to two 128x128 triangles
                # applied multiplicatively on the exp'd tiles (DVE), off the
                # PE. delta = qc*QC - si*128 selects the pattern:
                #   0     diagonal: cols 0:128 get tri_lo (t>=s)
                #   -128  diagonal: cols 0:128 are dead (scores restricted to
                #         128:256, low half memset 0), band tri_lo on 128:256
                #   896   window edge: band tri_hi (t<s) on cols 128:256
                #   1024  window edge: only cols 0:128 live, band tri_hi
                # Consume (sum/av) matmuls are full width except delta=1024;
                # consume order puts the full-width 896 tile first so the
                # psum accumulation anchor (start=True) covers all columns.
                for qc in range(N_QC):
                    qsl = slice(qc * QC, (qc + 1) * QC)
                    si_lo = max(0, 2 * qc - 8)
                    sis = list(range(si_lo, 2 * qc + 2))
                    cons = list(sis)
                    for hp in range(0, HL, 2):
                        avs = psB_p.tile([128, 2, QC], F32, tag="av", name="av")
                        sums = psC_p.tile([1, 2, QC], F32, tag="ts", name="sum")
                        exs = {}

                        # the two heads of a pair share kT/vnat, so their q
                        # columns ride one matmul stream (512 free): halves
                        # the PE instruction count and the ACT op count
                        def emit_scores(si):
                            delta = qc * QC - si * 128
                            psc = ps512_p.tile([128, 2, QC], F32, tag="sc", name="psc")
                            nc.tensor.matmul(
                                psc[:], kT[:, si * 128:(si + 1) * 128],
                                qT[:, hp:hp + 2, qsl],
                                start=True, stop=True,
                            )
                            ex = exp_p.tile([128, 2, QC], BF16, tag="ex")
                            nc.scalar.activation(ex[:], psc[:], AF.Exp)
                            for j in range(2):
                                if delta == 0:
                                    nc.vector.tensor_mul(
                                        ex[:, j, 0:128], ex[:, j, 0:128],
                                        mask_sb[:, 0, 128:256])
                                elif delta == -128:
                                    nc.vector.tensor_mul(
                                        ex[:, j, :], ex[:, j, :], mask_sb[:, 0, :])
                                elif delta == 896:
                                    nc.vector.tensor_mul(
                                        ex[:, j, 128:256], ex[:, j, 128:256],
                                        mask_sb[:, 1, 0:128])
                                elif delta == 1024:
                                    nc.vector.tensor_mul(
                                        ex[:, j, :], ex[:, j, :], mask_sb[:, 1, :])
                            exs[si] = ex

                        def emit_consume(si):
                            first = si == cons[0]
                            last = si == cons[-1]
                            ex = exs.pop(si)
                            nc.tensor.matmul(
                                sums[:], onc[:], ex[:],
                                start=first, stop=last,
                            )
                            nc.tensor.matmul(
                                avs[:], vnat[:, si, :], ex[:],
                                start=first, stop=last,
                            )

                        stag = min(4, len(sis))
                        for si in sis[:stag]:
                            emit_scores(si)
                        flush_norm2()
                        if hp == 0:
                            # previous slab's o_proj + RS, deferred here so
                            # the PE rides through the slab boundary on this
                            # slab's first score tiles
                            while oproj_q:
                                emit_oproj_group()
                        for idx, si in enumerate(sis[stag:]):
                            emit_scores(si)
                            emit_consume(cons[idx])
                        for k in range(stag, 0, -1):
                            emit_consume(cons[-k])

                        def make_norm(hp=hp, avs=avs, sums=sums, qsl=qsl):
                            def _norm():
                                sc_s = stB.tile([1, 2, QC], F32R, tag="rc", name="rc")
                                nc.scalar.activation(sc_s[:], sums[:], AF.Copy)
                                bcp = ps512_p.tile([128, 2, QC], F32, tag="sc", name="bcB")
                                nc.tensor.matmul(bcp[:], onr[:], sc_s[:], start=True, stop=True)
                                rcw = stB.tile([128, 2, QC], F32, tag="rcw", name="rcw")
                                nc.vector.reciprocal(rcw[:], bcp[:])
                                nc.vector.tensor_mul(
                                    attnT[:, hp:hp + 2, qsl], avs[:], rcw[:]
                                )
                            return _norm

                        pending_norm[0] = make_norm()
                    if qc >= 3:
                        flush_norm()
                        queue_oproj(qc)
                        while oproj_q:
                            emit_oproj_group()
                    else:
                        queue_oproj(qc)

    nc.finalize()
    return nc


_NC_CACHE = None


def _get_nc():
    global _NC_CACHE
    if _NC_CACHE is None:
        _NC_CACHE = _build()
    return _NC_CACHE


def _host_inputs(positions, hidden_states, w_qkv, q_norm_w, k_norm_w, w_o):
    positions = np.asarray(positions)
    hidden_states = np.asarray(hidden_states, dtype=np.float32)
    w_qkv = np.asarray(w_qkv, dtype=np.float32)
    q_norm_w = np.asarray(q_norm_w, dtype=np.float32)
    k_norm_w = np.asarray(k_norm_w, dtype=np.float32)
    w_o = np.asarray(w_o, dtype=np.float32)

    # [ki, tt, ko, j]
    hidT4 = np.ascontiguousarray(
        hidden_states.T.reshape(KO, 128, NT, 128).transpose(1, 2, 0, 3)
    ).astype(ml_dtypes.bfloat16)

    half = D // 2
    inv_freq = 1.0 / (THETA ** (np.arange(half, dtype=np.float32) / half))
    ang = positions.astype(np.float32)[:, None] * inv_freq[None, :]  # [T, 64]
    cos = np.cos(ang).T.astype(np.float32)   # [64, T]
    sin = np.sin(ang).T.astype(np.float32)
    cs2 = np.concatenate([cos, cos], axis=0)          # [128, T]
    sn2s = np.concatenate([-sin, sin], axis=0)        # [128, T]

    ss = np.arange(128)[:, None]
    ttv = np.arange(128)[None, :]
    tri_lo = (ttv >= ss).astype(np.float32)   # valid iff t >= s
    tri_hi = (ttv < ss).astype(np.float32)    # valid iff t < s
    zer = np.zeros_like(tri_lo)
    mk = np.ascontiguousarray(
        np.stack([
            np.concatenate([zer, tri_lo], axis=1),   # delta -128 full mask
            np.concatenate([tri_hi, zer], axis=1),   # delta 1024 full mask
        ], axis=1)
    ).astype(ml_dtypes.bfloat16)              # [128, 2, 256]

    qwv = (q_norm_w * (D ** -0.5)).reshape(D, 1).astype(np.float32)
    kwv = k_norm_w.reshape(D, 1).astype(np.float32)
    onc = np.ones((128, 1), ml_dtypes.bfloat16)
    onr = np.ones((1, 128), np.float32)

    in_maps = []
    for c in range(N_CORES):
        wq_c = np.concatenate(
            [
                w_qkv[:, c * HL * D:(c + 1) * HL * D],
                w_qkv[:, H * D + c * D:H * D + (c + 1) * D],
                w_qkv[:, (H + HKV) * D + c * D:(H + HKV) * D + (c + 1) * D],
            ],
            axis=1,
        )
        wq_c = np.ascontiguousarray(wq_c.reshape(KO, 128, (HL + 2) * D).transpose(1, 0, 2)).astype(ml_dtypes.bfloat16)
        wo_c = np.ascontiguousarray(
            w_o[c * HL * D:(c + 1) * HL * D, :].reshape(HL, 128, HID).transpose(1, 0, 2)
        ).astype(ml_dtypes.bfloat16)
        in_maps.append(
            {
                "hidT": hidT4,
                "wq": wq_c,
                "wo": wo_c,
                "cs2": cs2,
                "sn2s": sn2s,
                "masks": mk,
                "qwv": qwv,
                "kwv": kwv,
                "onc": onc,
                "onr": onr,
            }
        )
    return in_maps


def _assemble(results):
    out = np.empty((T, HID), np.float32)
    rows = QC // N_CORES
    half = rows // 2
    for c in range(N_CORES):
        r = np.asarray(results[c]["out"], dtype=np.float32)  # [N_QC, rows, HID]
        for qc in range(N_QC - 1):
            out[qc * QC + c * rows: qc * QC + (c + 1) * rows] = r[qc]
        # final slab was reduce-scattered as two 128-row halves
        qc = N_QC - 1
        base = qc * QC
        out[base + c * half: base + (c + 1) * half] = r[qc][:half]
        out[base + QC // 2 + c * half: base + QC // 2 + (c + 1) * half] = r[qc][half:]
    return out


def run_spmd(in_maps, trace=False, **kw):
    nc = _get_nc()
    return run_bass_kernel_spmd(nc, in_maps, list(range(N_CORES)), trace=trace, **kw)


def kernel(positions, hidden_states, w_qkv, q_norm_w, k_norm_w, w_o):
    in_maps = _host_inputs(positions, hidden_states, w_qkv, q_norm_w, k_norm_w, w_o)
    last_err = None
    for _ in range(3):
        try:
            res = run_spmd(in_maps)
            return _assemble(res.results)
        except Exception as e:  # rare transient NRT_EXEC_UNIT_UNRECOVERABLE
            last_err = e
    raise last_err


# revision 42
# speedup vs baseline: 1.0206x; 1.0206x over previous
"""Bass/Trainium2 kernel for nn_ExaoneMoEAttention (sliding-window GQA attention).

Strategy (8 NeuronCores, tensor-parallel over heads):
  - core c owns q heads 4c..4c+3 and kv head c (w_qkv column shard [4096, 768]),
    plus w_o rows 512c..512c+512 ([512, 4096]); hidden replicated.
  - Phase A (QKV proj): per 128-row t-tile, hidT tiles are the stationary
    operand and w_qkv columns the moving operand; RMSNorm stats via ACT
    Square+accum_out; normalized q/k tiles are PE-transposed to [d, t] strips
    (norm weight and softmax scale folded into the evacuation); RoPE via host
    cos/sin tables. The first two t-tiles are emitted w-chunk-major so the PE
    starts as soon as hid0 + w-chunk0 land and rides the weight stream.
  - Phase B: scoresT[s, q] tiles; 256-wide q chunks prune the causal/window
    block-sparsity at 128-key granularity (<=10 key tiles per chunk). The two
    heads of a GQA pair share kT/vnat and ride one matmul stream (512 free
    cols via a [d, 2, 256] AP). Softmax without max-subtraction; masking is
    multiplicative on the exp'd tiles (DVE) using two triangle patterns -
    every diagonal/window-edge mask reduces to [zeros|tri_lo] or
    [tri_hi|zeros] slices. Sums via ones-matmul; per-pair normalization by
    PE-broadcast reciprocal, deferred into the next pair's score stream.
    PSUM pools are shared across phases (same tags) so the phase boundary
    carries no pool-release stall; o_proj PSUM is evacuated on the Scalar
    engine to keep DVE off the critical chain.
  - o_proj per 256-row slab -> bf16 ReduceScatter(add) over 8 cores per slab
    (collectives are latency-floor bound, ~16-40us each, and serialize on
    the CC cores; draining each slab's o_proj immediately keeps every RS
    trigger early, which empirically halves per-RS time by avoiding
    queue-compounded cross-core skew at the end of the run). The
    final slab reduce-scatters as two 128-row halves so only ~one
    latency-floor op hangs past the last matmul. Host concatenates the
    row-shards.
"""

import ml_dtypes
import numpy as np

import concourse.mybir as mybir
import concourse.tile as tile
from concourse import bacc
from concourse.bass_utils import run_bass_kernel_spmd
from concourse.masks import make_identity

F32 = mybir.dt.float32
F32R = mybir.dt.float32r
BF16 = mybir.dt.bfloat16
AF = mybir.ActivationFunctionType

N_CORES = 8
T = 2048
HID = 4096
H = 32
HKV = 8
D = 128
EPS = 1e-5
THETA = 1e6
WINDOW = 1024

HL = H // N_CORES          # 4 local q heads
NT = T // 128              # 16 t/s tiles
KO = HID // 128            # 32 k-subtiles in projection
QC = 256                   # q chunk in attention phase
N_QC = T // QC             # 8
ECH = 512                  # o_proj e-chunk
N_ECH = HID // ECH         # 8


def _build():
    nc = bacc.Bacc(num_devices=N_CORES)

    # hidT4[ki, tt, ko, j] = hidden[tt*128+j, ko*128+ki]
    hidT = nc.declare_dram_parameter("hidT", [128, NT, KO, 128], BF16, isOutput=False)
    wq = nc.declare_dram_parameter("wq", [128, KO, (HL + 2) * D], BF16, isOutput=False)
    wo = nc.declare_dram_parameter("wo", [128, HL, HID], BF16, isOutput=False)
    cs2 = nc.declare_dram_parameter("cs2", [128, T], F32, isOutput=False)
    sn2s = nc.declare_dram_parameter("sn2s", [128, T], F32, isOutput=False)
    masks = nc.declare_dram_parameter("masks", [128, 2, 256], BF16, isOutput=False)
    qwv = nc.declare_dram_parameter("qwv", [D, 1], F32, isOutput=False)
    kwv = nc.declare_dram_parameter("kwv", [D, 1], F32, isOutput=False)
    onc_d = nc.declare_dram_parameter("onc", [128, 1], BF16, isOutput=False)
    onr_d = nc.declare_dram_parameter("onr", [1, 128], F32R, isOutput=False)
    out_p = nc.declare_dram_parameter("out", [N_QC, QC // N_CORES, HID], BF16, isOutput=True)

    with tile.TileContext(nc) as tc:
        with (
            tc.tile_pool(name="persistA", bufs=1) as pA,
            # PSUM pools span both phases (shared tags) — closing/reopening
            # psum pools at the phase boundary would stall Phase B's first
            # tiles on Phase A's full drain via released-zone deps.
            # ps512: A pq + B scores/o_proj/bcast (4x2KB); psB: A pq2 +
            # B av (3x2KB); psC: A transposes + B sums (1x2KB). 16KB total.
            tc.tile_pool(name="ps512", bufs=4, space="PSUM") as ps512_p,
            tc.tile_pool(name="psB", bufs=3, space="PSUM") as psB_p,
            tc.tile_pool(name="psC", bufs=1, space="PSUM") as psC_p,
        ):
            kT = pA.tile([128, T], F32R)                     # rope'd k, [d, s]
            qT = pA.tile([128, HL, T], F32R)                 # rope'd q, [d, h, t]
            vnat = pA.tile([128, NT, D], BF16)               # v in [s, d] tiles
            onc = pA.tile([128, 1], BF16)
            onr = pA.tile([1, 128], F32R)
            ident = pA.tile([128, 128], BF16)
            # Phase-B persistent tiles live here so their DMAs can be issued
            # mid-Phase-A (off the critical startup window)
            wo_sb = pA.tile([128, HL, HID], BF16)
            mask_sb = pA.tile([128, 2, 256], BF16)  # [zeros|tri_lo], [tri_hi|zeros]
            make_identity(nc, ident[:])
            nc.sync.dma_start(out=onc[:], in_=onc_d[:])
            nc.sync.dma_start(out=onr[:], in_=onr_d[:])

            # ---------------- Phase A: QKV projection + norm + rope ----------
            with (
                tc.tile_pool(name="wpool", bufs=1) as wpool,
                tc.tile_pool(name="hidp", bufs=4) as hidp,
                tc.tile_pool(name="cspool", bufs=4) as cspool,
                tc.tile_pool(name="tmpA", bufs=6) as tmpA,
                tc.tile_pool(name="stA", bufs=6) as stA,
                tc.tile_pool(name="miscA", bufs=1) as miscA,
            ):
                # DMA issue order shapes the startup ramp: the first matmuls
                # need only hid tile 0 + the first w chunk, so those go first
                # and the weight stream is split into 8 chunks with early hid
                # tiles interleaved (rope tables ride the gpsimd queue).
                prefetched = []

                def fetch_tt(tt):
                    tsl = slice(tt * 128, (tt + 1) * 128)
                    hid_t = hidp.tile([128, KO, 128], BF16, tag="hid")
                    nc.sync.dma_start(out=hid_t[:], in_=hidT[:, tt])
                    cs_t = cspool.tile([128, 128], F32, tag="cs")
                    sn_t = cspool.tile([128, 128], F32, tag="sn")
                    nc.gpsimd.dma_start(out=cs_t[:], in_=cs2[:, tsl])
                    nc.gpsimd.dma_start(out=sn_t[:], in_=sn2s[:, tsl])
                    prefetched.append((hid_t, cs_t, sn_t))

                NW = 8
                KW = KO // NW
                fetch_tt(0)
                w_grp = []
                for g in range(NW):
                    wt = wpool.tile([128, KW, (HL + 2) * D], BF16, name=f"w{g}")
                    nc.sync.dma_start(out=wt[:], in_=wq[:, g * KW:(g + 1) * KW, :])
                    w_grp.append(wt)
                    if g == 0:
                        fetch_tt(1)
                    elif g == 2:
                        fetch_tt(2)
                qw_sb = miscA.tile([D, 1], F32)
                kw_sb = miscA.tile([D, 1], F32)
                eps_sb = miscA.tile([128, 1], F32)
                nc.sync.dma_start(out=qw_sb[:], in_=qwv[:])
                nc.sync.dma_start(out=kw_sb[:], in_=kwv[:])
                nc.vector.memset(eps_sb[:], EPS)

                pending_post = []

                def flush_post(keep=0):
                    while len(pending_post) > keep:
                        pending_post.pop(0)()

                def make_post(tt, pq, pq2, tsl, cs_t, sn_t):
                    def _post():
                        # v copy first so pq2 drains early; m=0 is the k head
                        # (also pq2) so its chain leads each stage
                        nc.vector.tensor_copy(vnat[:, tt, :], pq2[:, D:2 * D])
                        # stage-parallel across the 5 normed heads so the
                        # ACT/DVE chains pipeline instead of serializing
                        srcs = [pq2[:, 0:D]] + [pq[:, m * D:(m + 1) * D] for m in range(HL)]
                        var, sd, rstd, ev, tp, qd, qsw = [], [], [], [], [], [], []
                        for m in range(HL + 1):
                            sqd = tmpA.tile([128, D], F32, tag="sqd", name="sqd")
                            var.append(stA.tile([128, 1], F32, tag="var", name="var"))
                            nc.scalar.activation(sqd[:], srcs[m], AF.Square, accum_out=var[m][:])
                        for m in range(HL + 1):
                            sd.append(stA.tile([128, 1], F32, tag="sd", name="sd"))
                            nc.scalar.activation(sd[m][:], var[m][:], AF.Sqrt, scale=1.0 / D, bias=eps_sb[:])
                        for m in range(HL + 1):
                            rstd.append(stA.tile([128, 1], F32, tag="rstd", name="rstd"))
                            nc.vector.reciprocal(rstd[m][:], sd[m][:])
                        for m in range(HL + 1):
                            ev.append(tmpA.tile([128, D], BF16, tag="ev", name="ev"))
                            nc.scalar.activation(ev[m][:], srcs[m], AF.Copy, scale=rstd[m][:])
                        for m in range(HL + 1):
                            tp.append(psC_p.tile([128, 128], BF16, tag="ts", name="tp"))
                            nc.tensor.transpose(tp[m][:], ev[m][:], ident[:])
                        for m in range(HL + 1):
                            qd.append(tmpA.tile([128, D], F32, tag="qd", name="qd"))
                            nc.scalar.activation(
                                qd[m][:], tp[m][:], AF.Copy,
                                scale=(kw_sb[:] if m == 0 else qw_sb[:]),
                            )
                        for m in range(HL + 1):
                            qsw.append(tmpA.tile([128, D], F32, tag="qsw", name="qsw"))
                            nc.vector.tensor_copy(qsw[m][0:64, :], qd[m][64:128, :])
                            nc.vector.tensor_copy(qsw[m][64:128, :], qd[m][0:64, :])
                        for m in range(HL + 1):
                            nc.vector.tensor_mul(qd[m][:], qd[m][:], cs_t[:])
                            nc.vector.tensor_mul(qsw[m][:], qsw[m][:], sn_t[:])
                            dst = kT[:, tsl] if m == 0 else qT[:, m - 1, tsl]
                            nc.vector.tensor_add(dst, qd[m][:], qsw[m][:])
                    return _post

                def emit_tile_mms(tt, pq, pq2, hid_t, g_range):
                    for g in g_range:
                        for k in range(KW):
                            ko = g * KW + k
                            lhsT = hid_t[:, ko, :]
                            nc.tensor.matmul(
                                pq[:], lhsT, w_grp[g][:, k, 0:4 * D],
                                start=(ko == 0), stop=(ko == KO - 1),
                            )
                            nc.tensor.matmul(
                                pq2[:], lhsT, w_grp[g][:, k, 4 * D:6 * D],
                                start=(ko == 0), stop=(ko == KO - 1),
                            )

                # ramp: tiles 0-2 emitted w-chunk-major (PE starts on chunk 0;
                # 3 tiles x 1.28us per w chunk outpaces the ~3.1us chunk
                # arrival so the PE saturates through the weight stream)
                RAMP = 3
                ramp = []
                for tt in range(RAMP):
                    pq = ps512_p.tile([128, 4 * D], F32, tag="sc", name=f"pqr{tt}")
                    pq2 = psB_p.tile([128, 2 * D], F32, tag="av", name=f"pq2r{tt}")
                    ramp.append((pq, pq2))
                for g in range(NW):
                    for tt in range(RAMP):
                        emit_tile_mms(tt, ramp[tt][0], ramp[tt][1], prefetched[tt][0], [g])
                for tt in range(RAMP):
                    hid_t, cs_t, sn_t = prefetched.pop(0)
                    tsl = slice(tt * 128, (tt + 1) * 128)
                    pending_post.append(make_post(tt, ramp[tt][0], ramp[tt][1], tsl, cs_t, sn_t))

                next_fetch = 3  # tiles 0-2 are always fetched pre-loop
                for tt in range(RAMP, NT):
                    tsl = slice(tt * 128, (tt + 1) * 128)
                    while next_fetch < min(tt + 4, NT):
                        fetch_tt(next_fetch)
                        next_fetch += 1
                    hid_t, cs_t, sn_t = prefetched.pop(0)
                    if tt == RAMP + 2:
                        # issued only after the w stream has drained — these
                        # share DMA engines with the sync queue and would
                        # dilute the startup weight ramp otherwise
                        nc.gpsimd.dma_start(out=mask_sb[:], in_=masks[:])
                    elif tt == RAMP + 4:
                        nc.gpsimd.dma_start(out=wo_sb[:], in_=wo[:])

                    # qkv[t, c] for this t-tile: [128, 512] + [128, 256] psums
                    pq = ps512_p.tile([128, 4 * D], F32, tag="sc", name="pq")
                    pq2 = psB_p.tile([128, 2 * D], F32, tag="av", name="pq2")
                    emit_tile_mms(tt, pq, pq2, hid_t, range(NW))
                    flush_post(keep=1)
                    pending_post.append(make_post(tt, pq, pq2, tsl, cs_t, sn_t))
                flush_post()

            # ---------------- Phase B: attention + o_proj + reduce-scatter ---
            with (
                tc.tile_pool(name="persistB", bufs=1) as pB,
                tc.tile_pool(name="exp", bufs=8) as exp_p,
                tc.tile_pool(name="stB", bufs=2) as stB,
                tc.tile_pool(name="ostg", bufs=8) as ostg_p,
                tc.tile_pool(name="dramB", bufs=1, space="DRAM") as dramB,
            ):
                attnT = pB.tile([128, HL, T], BF16)

                partial = [
                    dramB.tile([QC, HID], BF16, name=f"partial{qc}") for qc in range(N_QC)
                ]
                rs_out = [
                    dramB.tile([QC // N_CORES, HID], BF16, name=f"rsout{qc}")
                    for qc in range(N_QC)
                ]
                rs_half = [
                    dramB.tile([QC // (2 * N_CORES), HID], BF16, name=f"rshalf{i}")
                    for i in range(2)
                ]

                pending_norm = [None]

                def flush_norm():
                    if pending_norm[0] is not None:
                        pending_norm[0]()
                        pending_norm[0] = None

                flush_norm2 = flush_norm

                oproj_q = []

                def emit_oproj_group():
                    if oproj_q:
                        oproj_q.pop(0)()

                def queue_oproj(qc):
                    def emit_rs(qc, half=None):
                        if half is None:
                            nc.gpsimd.collective_compute(
                                "ReduceScatter",
                                mybir.AluOpType.add,
                                replica_groups=[list(range(N_CORES))],
                                ins=[partial[qc][:]],
                                outs=[rs_out[qc][:]],
                            )
                            # same gpsimd queue as the RS, so this wait can't
                            # poison other engines
                            nc.gpsimd.dma_start(out=out_p[qc], in_=rs_out[qc][:])
                        else:
                            # final slab: two 128-row (1MB) halves so the RS
                            # past the last matmul is one latency-floor op
                            nc.gpsimd.collective_compute(
                                "ReduceScatter",
                                mybir.AluOpType.add,
                                replica_groups=[list(range(N_CORES))],
                                ins=[partial[qc][half * 128:(half + 1) * 128, :]],
                                outs=[rs_half[half][:]],
                            )
                            ho = QC // (2 * N_CORES)
                            nc.gpsimd.dma_start(
                                out=out_p[qc, half * ho:(half + 1) * ho, :],
                                in_=rs_half[half][:],
                            )

                    def make_group(trow, ec, rs_after):
                        def _g():
                            pso = ps512_p.tile([128, ECH], F32, tag="sc", name="pso")
                            for h in range(HL):
                                nc.tensor.matmul(
                                    pso[:],
                                    attnT[:, h, trow * 128:(trow + 1) * 128],
                                    wo_sb[:, h, ec * ECH:(ec + 1) * ECH],
                                    start=(h == 0),
                                    stop=(h == HL - 1),
                                )
                            ost = ostg_p.tile([128, ECH], BF16, tag="ost", name="ost")
                            # alternate evacuation engine to balance ACT/DVE
                            if ec % 2 == 0:
                                nc.scalar.activation(ost[:], pso[:], AF.Copy)
                            else:
                                nc.vector.tensor_copy(ost[:], pso[:])
                            nc.sync.dma_start(
                                out=partial[qc][(trow % 2) * 128:(trow % 2 + 1) * 128,
                                                ec * ECH:(ec + 1) * ECH],
                                in_=ost[:],
                            )
                            if rs_after is not None:
                                emit_rs(qc, None if rs_after < 0 else rs_after)
                        return _g

                    rows = [qc * 2 + tt for tt in range(QC // 128)]
                    last_slab = qc == N_QC - 1
                    i = 0
                    for qi, trow in enumerate(rows):
                        for ec in range(N_ECH):
                            i += 1
                            rs_after = None
                            if last_slab and i % N_ECH == 0:
                                rs_after = qi
                            elif i == len(rows) * N_ECH:
                                rs_after = -1
                            oproj_q.append(make_group(trow, ec, rs_after))

                # Per 256-wide q chunk, key tiles si run from the window edge
                # to the diagonal. Masking reduces to two 128x128 triangles
                # applied multiplicatively on the exp'd tiles (DVE), off the
                # PE. delta = qc*QC - si*128 selects the pattern:
                #   0     diagonal: cols 0:128 get tri_lo (t>=s)
                #   -128  diagonal: cols 0:128 are dead (scores restricted to
                #         128:256, low half memset 0), band tri_lo on 128:256
                #   896   window edge: band tri_hi (t<s) on cols 128:256
                #   1024  window edge: only cols 0:128 live, band tri_hi
                # Consume (sum/av) matmuls are full width except delta=1024;
                # consume order puts the full-width 896 tile first so the
                # psum accumulation anchor (start=True) covers all columns.
                for qc in range(N_QC):
                    qsl = slice(qc * QC, (qc + 1) * QC)
                    si_lo = max(0, 2 * qc - 8)
                    sis = list(range(si_lo, 2 * qc + 2))
                    cons = list(sis)
                    for hp in range(0, HL, 2):
                        avs = psB_p.tile([128, 2, QC], F32, tag="av", name="av")
                        sums = psC_p.tile([1, 2, QC], F32, tag="ts", name="sum")
                        exs = {}

                        # the two heads of a pair share kT/vnat, so their q
                        # columns ride one matmul stream (512 free): halves
                        # the PE instruction count and the ACT op count
                        def emit_scores(si):
                            delta = qc * QC - si * 128
                            psc = ps512_p.tile([128, 2, QC], F32, tag="sc", name="psc")
                            nc.tensor.matmul(
                                psc[:], kT[:, si * 128:(si + 1) * 128],
                                qT[:, hp:hp + 2, qsl],
                                start=True, stop=True,
                            )
                            ex = exp_p.tile([128, 2, QC], BF16, tag="ex")
                            nc.scalar.activation(ex[:], psc[:], AF.Exp)
                            for j in range(2):
                                if delta == 0:
                                    nc.vector.tensor_mul(
                                        ex[:, j, 0:128], ex[:, j, 0:128],
                                        mask_sb[:, 0, 128:256])
                                elif delta == -128:
                                    nc.vector.tensor_mul(
                                        ex[:, j, :], ex[:, j, :], mask_sb[:, 0, :])
                                elif delta == 896:
                                    nc.vector.tensor_mul(
                                        ex[:, j, 128:256], ex[:, j, 128:256],
                                        mask_sb[:, 1, 0:128])
                                elif delta == 1024:
                                    nc.vector.tensor_mul(
                                        ex[:, j, :], ex[:, j, :], mask_sb[:, 1, :])
                            exs[si] = ex

                        def emit_consume(si):
                            first = si == cons[0]
                            last = si == cons[-1]
                            ex = exs.pop(si)
                            nc.tensor.matmul(
                                sums[:], onc[:], ex[:],
                                start=first, stop=last,
                            )
                            nc.tensor.matmul(
                                avs[:], vnat[:, si, :], ex[:],
                                start=first, stop=last,
                            )

                        stag = min(4, len(sis))
                        for si in sis[:stag]:
                            emit_scores(si)
                        flush_norm2()
                        if hp == 0:
                            # previous slab's o_proj + RS, deferred here so
                            # the PE rides through the slab boundary on this
                            # slab's first score tiles
                            while oproj_q:
                                emit_oproj_group()
                        for idx, si in enumerate(sis[stag:]):
                            emit_scores(si)
                            emit_consume(cons[idx])
                        for k in range(stag, 0, -1):
                            emit_consume(cons[-k])

                        def make_norm(hp=hp, avs=avs, sums=sums, qsl=qsl):
                            def _norm():
                                sc_s = stB.tile([1, 2, QC], F32R, tag="rc", name="rc")
                                nc.scalar.activation(sc_s[:], sums[:], AF.Copy)
                                bcp = ps512_p.tile([128, 2, QC], F32, tag="sc", name="bcB")
                                nc.tensor.matmul(bcp[:], onr[:], sc_s[:], start=True, stop=True)
                                rcw = stB.tile([128, 2, QC], F32, tag="rcw", name="rcw")
                                nc.vector.reciprocal(rcw[:], bcp[:])
                                nc.vector.tensor_mul(
                                    attnT[:, hp:hp + 2, qsl], avs[:], rcw[:]
                                )
                            return _norm

                        pending_norm[0] = make_norm()
                    if qc >= 0:
                        flush_norm()
                        queue_oproj(qc)
                        while oproj_q:
                            emit_oproj_group()
                    else:
                        queue_oproj(qc)

    nc.finalize()
    return nc


_NC_CACHE = None


def _get_nc():
    global _NC_CACHE
    if _NC_CACHE is None:
        _NC_CACHE = _build()
    return _NC_CACHE


def _host_inputs(positions, hidden_states, w_qkv, q_norm_w, k_norm_w, w_o):
    positions = np.asarray(positions)
    hidden_states = np.asarray(hidden_states, dtype=np.float32)
    w_qkv = np.asarray(w_qkv, dtype=np.float32)
    q_norm_w = np.asarray(q_norm_w, dtype=np.float32)
    k_norm_w = np.asarray(k_norm_w, dtype=np.float32)
    w_o = np.asarray(w_o, dtype=np.float32)

    # [ki, tt, ko, j]
    hidT4 = np.ascontiguousarray(
        hidden_states.T.reshape(KO, 128, NT, 128).transpose(1, 2, 0, 3)
    ).astype(ml_dtypes.bfloat16)

    half = D // 2
    inv_freq = 1.0 / (THETA ** (np.arange(half, dtype=np.float32) / half))
    ang = positions.astype(np.float32)[:, None] * inv_freq[None, :]  # [T, 64]
    cos = np.cos(ang).T.astype(np.float32)   # [64, T]
    sin = np.sin(ang).T.astype(np.float32)
    cs2 = np.concatenate([cos, cos], axis=0)          # [128, T]
    sn2s = np.concatenate([-sin, sin], axis=0)        # [128, T]

    ss = np.arange(128)[:, None]
    ttv = np.arange(128)[None, :]
    tri_lo = (ttv >= ss).astype(np.float32)   # valid iff t >= s
    tri_hi = (ttv < ss).astype(np.float32)    # valid iff t < s
    zer = np.zeros_like(tri_lo)
    mk = np.ascontiguousarray(
        np.stack([
            np.concatenate([zer, tri_lo], axis=1),   # delta -128 full mask
            np.concatenate([tri_hi, zer], axis=1),   # delta 1024 full mask
        ], axis=1)
    ).astype(ml_dtypes.bfloat16)              # [128, 2, 256]

    qwv = (q_norm_w * (D ** -0.5)).reshape(D, 1).astype(np.float32)
    kwv = k_norm_w.reshape(D, 1).astype(np.float32)
    onc = np.ones((128, 1), ml_dtypes.bfloat16)
    onr = np.ones((1, 128), np.float32)

    in_maps = []
    for c in range(N_CORES):
        wq_c = np.concatenate(
            [
                w_qkv[:, c * HL * D:(c + 1) * HL * D],
                w_qkv[:, H * D + c * D:H * D + (c + 1) * D],
                w_qkv[:, (H + HKV) * D + c * D:(H + HKV) * D + (c + 1) * D],
            ],
            axis=1,
        )
        wq_c = np.ascontiguousarray(wq_c.reshape(KO, 128, (HL + 2) * D).transpose(1, 0, 2)).astype(ml_dtypes.bfloat16)
        wo_c = np.ascontiguousarray(
            w_o[c * HL * D:(c + 1) * HL * D, :].reshape(HL, 128, HID).transpose(1, 0, 2)
        ).astype(ml_dtypes.bfloat16)
        in_maps.append(
            {
                "hidT": hidT4,
                "wq": wq_c,
                "wo": wo_c,
                "cs2": cs2,
                "sn2s": sn2s,
                "masks": mk,
                "qwv": qwv,
                "kwv": kwv,
                "onc": onc,
                "onr": onr,
            }
        )
    return in_maps


def _assemble(results):
    out = np.empty((T, HID), np.float32)
    rows = QC // N_CORES
    half = rows // 2
    for c in range(N_CORES):
        r = np.asarray(results[c]["out"], dtype=np.float32)  # [N_QC, rows, HID]
        for qc in range(N_QC - 1):
            out[qc * QC + c * rows: qc * QC + (c + 1) * rows] = r[qc]
        # final slab was reduce-scattered as two 128-row halves
        qc = N_QC - 1
        base = qc * QC
        out[base + c * half: base + (c + 1) * half] = r[qc][:half]
        out[base + QC // 2 + c * half: base + QC // 2 + (c + 1) * half] = r[qc][half:]
    return out


def run_spmd(in_maps, trace=False, **kw):
    nc = _get_nc()
    return run_bass_kernel_spmd(nc, in_maps, list(range(N_CORES)), trace=trace, **kw)


def kernel(positions, hidden_states, w_qkv, q_norm_w, k_norm_w, w_o):
    in_maps = _host_inputs(positions, hidden_states, w_qkv, q_norm_w, k_norm_w, w_o)
    last_err = None
    for _ in range(3):
        try:
            res = run_spmd(in_maps)
            return _assemble(res.results)
        except Exception as e:  # rare transient NRT_EXEC_UNIT_UNRECOVERABLE
            last_err = e
    raise last_err


# revision 43
# speedup vs baseline: 1.0298x; 1.0089x over previous
"""Bass/Trainium2 kernel for nn_ExaoneMoEAttention (sliding-window GQA attention).

Strategy (8 NeuronCores, tensor-parallel over heads):
  - core c owns q heads 4c..4c+3 and kv head c (w_qkv column shard [4096, 768]),
    plus w_o rows 512c..512c+512 ([512, 4096]); hidden replicated.
  - Phase A (QKV proj): per 128-row t-tile, hidT tiles are the stationary
    operand and w_qkv columns the moving operand; RMSNorm stats via ACT
    Square+accum_out; normalized q/k tiles are PE-transposed to [d, t] strips
    (norm weight and softmax scale folded into the evacuation); RoPE via host
    cos/sin tables. The first two t-tiles are emitted w-chunk-major so the PE
    starts as soon as hid0 + w-chunk0 land and rides the weight stream.
  - Phase B: scoresT[s, q] tiles; 256-wide q chunks prune the causal/window
    block-sparsity at 128-key granularity (<=10 key tiles per chunk). The two
    heads of a GQA pair share kT/vnat and ride one matmul stream (512 free
    cols via a [d, 2, 256] AP). Softmax without max-subtraction; masking is
    multiplicative on the exp'd tiles (DVE) using two triangle patterns -
    every diagonal/window-edge mask reduces to [zeros|tri_lo] or
    [tri_hi|zeros] slices. Sums via ones-matmul; per-pair normalization by
    PE-broadcast reciprocal, deferred into the next pair's score stream.
    PSUM pools are shared across phases (same tags) so the phase boundary
    carries no pool-release stall; o_proj PSUM is evacuated on the Scalar
    engine to keep DVE off the critical chain.
  - o_proj per 256-row slab -> bf16 ReduceScatter(add) over 8 cores per slab
    (collectives are latency-floor bound, ~16-40us each, and serialize on
    the CC cores; draining each slab's o_proj immediately keeps every RS
    trigger early, which empirically halves per-RS time by avoiding
    queue-compounded cross-core skew at the end of the run). The
    final slab reduce-scatters as two 128-row halves so only ~one
    latency-floor op hangs past the last matmul. Host concatenates the
    row-shards.
"""

import ml_dtypes
import numpy as np

import concourse.mybir as mybir
import concourse.tile as tile
from concourse import bacc
from concourse.bass_utils import run_bass_kernel_spmd
from concourse.masks import make_identity

F32 = mybir.dt.float32
F32R = mybir.dt.float32r
BF16 = mybir.dt.bfloat16
AF = mybir.ActivationFunctionType

N_CORES = 8
T = 2048
HID = 4096
H = 32
HKV = 8
D = 128
EPS = 1e-5
THETA = 1e6
WINDOW = 1024

HL = H // N_CORES          # 4 local q heads
NT = T // 128              # 16 t/s tiles
KO = HID // 128            # 32 k-subtiles in projection
QC = 256                   # q chunk in attention phase
N_QC = T // QC             # 8
ECH = 512                  # o_proj e-chunk
N_ECH = HID // ECH         # 8


def _build():
    nc = bacc.Bacc(num_devices=N_CORES)

    # hidT4[ki, tt, ko, j] = hidden[tt*128+j, ko*128+ki]
    hidT = nc.declare_dram_parameter("hidT", [128, NT, KO, 128], BF16, isOutput=False)
    wq = nc.declare_dram_parameter("wq", [128, KO, (HL + 2) * D], BF16, isOutput=False)
    wo = nc.declare_dram_parameter("wo", [128, HL, HID], BF16, isOutput=False)
    cs2 = nc.declare_dram_parameter("cs2", [128, T], F32, isOutput=False)
    sn2s = nc.declare_dram_parameter("sn2s", [128, T], F32, isOutput=False)
    masks = nc.declare_dram_parameter("masks", [128, 2, 256], BF16, isOutput=False)
    qwv = nc.declare_dram_parameter("qwv", [D, 1], F32, isOutput=False)
    kwv = nc.declare_dram_parameter("kwv", [D, 1], F32, isOutput=False)
    onc_d = nc.declare_dram_parameter("onc", [128, 1], BF16, isOutput=False)
    onr_d = nc.declare_dram_parameter("onr", [1, 128], F32R, isOutput=False)
    out_p = nc.declare_dram_parameter("out", [N_QC, QC // N_CORES, HID], BF16, isOutput=True)

    with tile.TileContext(nc) as tc:
        with (
            tc.tile_pool(name="persistA", bufs=1) as pA,
            # PSUM pools span both phases (shared tags) — closing/reopening
            # psum pools at the phase boundary would stall Phase B's first
            # tiles on Phase A's full drain via released-zone deps.
            # ps512: A pq + B scores/o_proj/bcast (4x2KB); psB: A pq2 +
            # B av (3x2KB); psC: A transposes + B sums (1x2KB). 16KB total.
            tc.tile_pool(name="ps512", bufs=4, space="PSUM") as ps512_p,
            tc.tile_pool(name="psB", bufs=3, space="PSUM") as psB_p,
            tc.tile_pool(name="psC", bufs=1, space="PSUM") as psC_p,
        ):
            kT = pA.tile([128, T], F32R)                     # rope'd k, [d, s]
            qT = pA.tile([128, HL, T], F32R)                 # rope'd q, [d, h, t]
            vnat = pA.tile([128, NT, D], BF16)               # v in [s, d] tiles
            onc = pA.tile([128, 1], BF16)
            onr = pA.tile([1, 128], F32R)
            ident = pA.tile([128, 128], BF16)
            # Phase-B persistent tiles live here so their DMAs can be issued
            # mid-Phase-A (off the critical startup window)
            wo_sb = pA.tile([128, HL, HID], BF16)
            mask_sb = pA.tile([128, 2, 256], BF16)  # [zeros|tri_lo], [tri_hi|zeros]
            make_identity(nc, ident[:])
            nc.sync.dma_start(out=onc[:], in_=onc_d[:])
            nc.sync.dma_start(out=onr[:], in_=onr_d[:])

            # ---------------- Phase A: QKV projection + norm + rope ----------
            with (
                tc.tile_pool(name="wpool", bufs=1) as wpool,
                tc.tile_pool(name="hidp", bufs=4) as hidp,
                tc.tile_pool(name="cspool", bufs=4) as cspool,
                tc.tile_pool(name="tmpA", bufs=6) as tmpA,
                tc.tile_pool(name="stA", bufs=6) as stA,
                tc.tile_pool(name="miscA", bufs=1) as miscA,
            ):
                # DMA issue order shapes the startup ramp: the first matmuls
                # need only hid tile 0 + the first w chunk, so those go first
                # and the weight stream is split into 8 chunks with early hid
                # tiles interleaved (rope tables ride the gpsimd queue).
                prefetched = []

                def fetch_tt(tt):
                    tsl = slice(tt * 128, (tt + 1) * 128)
                    hid_t = hidp.tile([128, KO, 128], BF16, tag="hid")
                    nc.sync.dma_start(out=hid_t[:], in_=hidT[:, tt])
                    cs_t = cspool.tile([128, 128], F32, tag="cs")
                    sn_t = cspool.tile([128, 128], F32, tag="sn")
                    nc.gpsimd.dma_start(out=cs_t[:], in_=cs2[:, tsl])
                    nc.gpsimd.dma_start(out=sn_t[:], in_=sn2s[:, tsl])
                    prefetched.append((hid_t, cs_t, sn_t))

                NW = 8
                KW = KO // NW
                fetch_tt(0)
                w_grp = []
                for g in range(NW):
                    wt = wpool.tile([128, KW, (HL + 2) * D], BF16, name=f"w{g}")
                    nc.sync.dma_start(out=wt[:], in_=wq[:, g * KW:(g + 1) * KW, :])
                    w_grp.append(wt)
                    if g == 0:
                        fetch_tt(1)
                    elif g == 2:
                        fetch_tt(2)
                qw_sb = miscA.tile([D, 1], F32)
                kw_sb = miscA.tile([D, 1], F32)
                eps_sb = miscA.tile([128, 1], F32)
                nc.sync.dma_start(out=qw_sb[:], in_=qwv[:])
                nc.sync.dma_start(out=kw_sb[:], in_=kwv[:])
                nc.vector.memset(eps_sb[:], EPS)

                pending_post = []

                def flush_post(keep=0):
                    while len(pending_post) > keep:
                        pending_post.pop(0)()

                def make_post(tt, pq, pq2, tsl, cs_t, sn_t):
                    def _post():
                        # v copy first so pq2 drains early; m=0 is the k head
                        # (also pq2) so its chain leads each stage
                        nc.vector.tensor_copy(vnat[:, tt, :], pq2[:, D:2 * D])
                        # stage-parallel across the 5 normed heads so the
                        # ACT/DVE chains pipeline instead of serializing
                        srcs = [pq2[:, 0:D]] + [pq[:, m * D:(m + 1) * D] for m in range(HL)]
                        var, sd, rstd, ev, tp, qd, qsw = [], [], [], [], [], [], []
                        for m in range(HL + 1):
                            sqd = tmpA.tile([128, D], F32, tag="sqd", name="sqd")
                            var.append(stA.tile([128, 1], F32, tag="var", name="var"))
                            nc.scalar.activation(sqd[:], srcs[m], AF.Square, accum_out=var[m][:])
                        for m in range(HL + 1):
                            sd.append(stA.tile([128, 1], F32, tag="sd", name="sd"))
                            nc.scalar.activation(sd[m][:], var[m][:], AF.Sqrt, scale=1.0 / D, bias=eps_sb[:])
                        for m in range(HL + 1):
                            rstd.append(stA.tile([128, 1], F32, tag="rstd", name="rstd"))
                            nc.vector.reciprocal(rstd[m][:], sd[m][:])
                        for m in range(HL + 1):
                            ev.append(tmpA.tile([128, D], BF16, tag="ev", name="ev"))
                            nc.scalar.activation(ev[m][:], srcs[m], AF.Copy, scale=rstd[m][:])
                        for m in range(HL + 1):
                            tp.append(psC_p.tile([128, 128], BF16, tag="ts", name="tp"))
                            nc.tensor.transpose(tp[m][:], ev[m][:], ident[:])
                        for m in range(HL + 1):
                            qd.append(tmpA.tile([128, D], F32, tag="qd", name="qd"))
                            nc.scalar.activation(
                                qd[m][:], tp[m][:], AF.Copy,
                                scale=(kw_sb[:] if m == 0 else qw_sb[:]),
                            )
                        for m in range(HL + 1):
                            qsw.append(tmpA.tile([128, D], F32, tag="qsw", name="qsw"))
                            nc.vector.tensor_copy(qsw[m][0:64, :], qd[m][64:128, :])
                            nc.vector.tensor_copy(qsw[m][64:128, :], qd[m][0:64, :])
                        for m in range(HL + 1):
                            nc.vector.tensor_mul(qd[m][:], qd[m][:], cs_t[:])
                            nc.vector.tensor_mul(qsw[m][:], qsw[m][:], sn_t[:])
                            dst = kT[:, tsl] if m == 0 else qT[:, m - 1, tsl]
                            nc.vector.tensor_add(dst, qd[m][:], qsw[m][:])
                    return _post

                def emit_tile_mms(tt, pq, pq2, hid_t, g_range):
                    for g in g_range:
                        for k in range(KW):
                            ko = g * KW + k
                            lhsT = hid_t[:, ko, :]
                            nc.tensor.matmul(
                                pq[:], lhsT, w_grp[g][:, k, 0:4 * D],
                                start=(ko == 0), stop=(ko == KO - 1),
                            )
                            nc.tensor.matmul(
                                pq2[:], lhsT, w_grp[g][:, k, 4 * D:6 * D],
                                start=(ko == 0), stop=(ko == KO - 1),
                            )

                # ramp: tiles 0-2 emitted w-chunk-major (PE starts on chunk 0;
                # 3 tiles x 1.28us per w chunk outpaces the ~3.1us chunk
                # arrival so the PE saturates through the weight stream)
                RAMP = 3
                ramp = []
                for tt in range(RAMP):
                    pq = ps512_p.tile([128, 4 * D], F32, tag="sc", name=f"pqr{tt}")
                    pq2 = psB_p.tile([128, 2 * D], F32, tag="av", name=f"pq2r{tt}")
                    ramp.append((pq, pq2))
                for g in range(NW):
                    for tt in range(RAMP):
                        emit_tile_mms(tt, ramp[tt][0], ramp[tt][1], prefetched[tt][0], [g])
                for tt in range(RAMP):
                    hid_t, cs_t, sn_t = prefetched.pop(0)
                    tsl = slice(tt * 128, (tt + 1) * 128)
                    pending_post.append(make_post(tt, ramp[tt][0], ramp[tt][1], tsl, cs_t, sn_t))

                next_fetch = 3  # tiles 0-2 are always fetched pre-loop
                for tt in range(RAMP, NT):
                    tsl = slice(tt * 128, (tt + 1) * 128)
                    while next_fetch < min(tt + 4, NT):
                        fetch_tt(next_fetch)
                        next_fetch += 1
                    hid_t, cs_t, sn_t = prefetched.pop(0)
                    if tt == RAMP + 2:
                        # issued only after the w stream has drained — these
                        # share DMA engines with the sync queue and would
                        # dilute the startup weight ramp otherwise
                        nc.gpsimd.dma_start(out=mask_sb[:], in_=masks[:])
                    elif tt == RAMP + 4:
                        nc.gpsimd.dma_start(out=wo_sb[:], in_=wo[:])

                    # qkv[t, c] for this t-tile: [128, 512] + [128, 256] psums
                    pq = ps512_p.tile([128, 4 * D], F32, tag="sc", name="pq")
                    pq2 = psB_p.tile([128, 2 * D], F32, tag="av", name="pq2")
                    emit_tile_mms(tt, pq, pq2, hid_t, range(NW))
                    flush_post(keep=1)
                    pending_post.append(make_post(tt, pq, pq2, tsl, cs_t, sn_t))
                flush_post()

            # ---------------- Phase B: attention + o_proj + reduce-scatter ---
            with (
                tc.tile_pool(name="persistB", bufs=1) as pB,
                tc.tile_pool(name="exp", bufs=8) as exp_p,
                tc.tile_pool(name="stB", bufs=2) as stB,
                tc.tile_pool(name="ostg", bufs=8) as ostg_p,
                tc.tile_pool(name="dramB", bufs=1, space="DRAM") as dramB,
            ):
                attnT = pB.tile([128, HL, T], BF16)

                partial = [
                    dramB.tile([QC, HID], BF16, name=f"partial{qc}") for qc in range(N_QC)
                ]
                rs_out = [
                    dramB.tile([QC // N_CORES, HID], BF16, name=f"rsout{qc}")
                    for qc in range(N_QC)
                ]
                rs_half = [
                    dramB.tile([QC // (2 * N_CORES), HID], BF16, name=f"rshalf{i}")
                    for i in range(2)
                ]

                pending_norm = [None]

                def flush_norm():
                    if pending_norm[0] is not None:
                        pending_norm[0]()
                        pending_norm[0] = None

                flush_norm2 = flush_norm

                oproj_q = []

                def emit_oproj_group():
                    if oproj_q:
                        oproj_q.pop(0)()

                def queue_oproj(qc):
                    def emit_rs(qc, half=None):
                        if half is None:
                            nc.gpsimd.collective_compute(
                                "ReduceScatter",
                                mybir.AluOpType.add,
                                replica_groups=[list(range(N_CORES))],
                                ins=[partial[qc][:]],
                                outs=[rs_out[qc][:]],
                            )
                            # same gpsimd queue as the RS, so this wait can't
                            # poison other engines
                            nc.gpsimd.dma_start(out=out_p[qc], in_=rs_out[qc][:])
                        else:
                            # final slab: two 128-row (1MB) halves so the RS
                            # past the last matmul is one latency-floor op
                            nc.gpsimd.collective_compute(
                                "ReduceScatter",
                                mybir.AluOpType.add,
                                replica_groups=[list(range(N_CORES))],
                                ins=[partial[qc][half * 128:(half + 1) * 128, :]],
                                outs=[rs_half[half][:]],
                            )
                            ho = QC // (2 * N_CORES)
                            nc.gpsimd.dma_start(
                                out=out_p[qc, half * ho:(half + 1) * ho, :],
                                in_=rs_half[half][:],
                            )

                    def make_group(trow, ec, rs_after):
                        def _g():
                            pso = ps512_p.tile([128, ECH], F32, tag="sc", name="pso")
                            for h in range(HL):
                                nc.tensor.matmul(
                                    pso[:],
                                    attnT[:, h, trow * 128:(trow + 1) * 128],
                                    wo_sb[:, h, ec * ECH:(ec + 1) * ECH],
                                    start=(h == 0),
                                    stop=(h == HL - 1),
                                )
                            ost = ostg_p.tile([128, ECH], BF16, tag="ost", name="ost")
                            # alternate evacuation engine to balance ACT/DVE
                            if ec % 2 == 0:
                                nc.scalar.activation(ost[:], pso[:], AF.Copy)
                            else:
                                nc.vector.tensor_copy(ost[:], pso[:])
                            nc.sync.dma_start(
                                out=partial[qc][(trow % 2) * 128:(trow % 2 + 1) * 128,
                                                ec * ECH:(ec + 1) * ECH],
                                in_=ost[:],
                            )
                            if rs_after is not None:
                                emit_rs(qc, None if rs_after < 0 else rs_after)
                        return _g

                    rows = [qc * 2 + tt for tt in range(QC // 128)]
                    last_slab = qc == N_QC - 1
                    i = 0
                    for qi, trow in enumerate(rows):
                        for ec in range(N_ECH):
                            i += 1
                            rs_after = None
                            if last_slab and i % N_ECH == 0:
                                rs_after = qi
                            elif i == len(rows) * N_ECH:
                                rs_after = -1
                            oproj_q.append(make_group(trow, ec, rs_after))

                # Per 256-wide q chunk, key tiles si run from the window edge
                # to the diagonal. Masking reduces to two 128x128 triangles
                # applied multiplicatively on the exp'd tiles (DVE), off the
                # PE. delta = qc*QC - si*128 selects the pattern:
                #   0     diagonal: cols 0:128 get tri_lo (t>=s)
                #   -128  diagonal: cols 0:128 are dead (scores restricted to
                #         128:256, low half memset 0), band tri_lo on 128:256
                #   896   window edge: band tri_hi (t<s) on cols 128:256
                #   1024  window edge: only cols 0:128 live, band tri_hi
                # Consume (sum/av) matmuls are full width except delta=1024;
                # consume order puts the full-width 896 tile first so the
                # psum accumulation anchor (start=True) covers all columns.
                for qc in range(N_QC):
                    qsl = slice(qc * QC, (qc + 1) * QC)
                    si_lo = max(0, 2 * qc - 8)
                    sis = list(range(si_lo, 2 * qc + 2))
                    cons = list(sis)
                    if qc == 0:
                        # both head-pairs' scores up front: independent PE
                        # work that rides over Phase A's final post-chain
                        # drain (keeps the A->B gap under the HAM threshold)
                        store = []
                        for hp in (0, 2):
                            avs = psB_p.tile([128, 2, QC], F32, tag="av", name="av")
                            sums = psC_p.tile([1, 2, QC], F32, tag="ts", name="sum")
                            exq = []
                            for si in sis:
                                psc = ps512_p.tile([128, 2, QC], F32, tag="sc", name="psc")
                                nc.tensor.matmul(
                                    psc[:], kT[:, si * 128:(si + 1) * 128],
                                    qT[:, hp:hp + 2, qsl], start=True, stop=True,
                                )
                                ex = exp_p.tile([128, 2, QC], BF16, tag="ex")
                                nc.scalar.activation(ex[:], psc[:], AF.Exp)
                                for j in range(2):
                                    if si == 0:      # delta 0: diagonal band
                                        nc.vector.tensor_mul(
                                            ex[:, j, 0:128], ex[:, j, 0:128],
                                            mask_sb[:, 0, 128:256])
                                    else:            # delta -128: full mask
                                        nc.vector.tensor_mul(
                                            ex[:, j, :], ex[:, j, :], mask_sb[:, 0, :])
                                exq.append(ex)
                            store.append((hp, avs, sums, exq))
                        for hp, avs, sums, exq in store:
                            for k, ex in enumerate(exq):
                                nc.tensor.matmul(
                                    sums[:], onc[:], ex[:], start=(k == 0), stop=(k == 1))
                                nc.tensor.matmul(
                                    avs[:], vnat[:, sis[k], :], ex[:],
                                    start=(k == 0), stop=(k == 1))
                            sc_s = stB.tile([1, 2, QC], F32R, tag="rc", name="rc")
                            nc.scalar.activation(sc_s[:], sums[:], AF.Copy)
                            bcp = ps512_p.tile([128, 2, QC], F32, tag="sc", name="bcB")
                            nc.tensor.matmul(bcp[:], onr[:], sc_s[:], start=True, stop=True)
                            rcw = stB.tile([128, 2, QC], F32, tag="rcw", name="rcw")
                            nc.vector.reciprocal(rcw[:], bcp[:])
                            nc.vector.tensor_mul(attnT[:, hp:hp + 2, qsl], avs[:], rcw[:])
                        queue_oproj(0)
                        while oproj_q:
                            emit_oproj_group()
                        continue
                    for hp in range(0, HL, 2):
                        avs = psB_p.tile([128, 2, QC], F32, tag="av", name="av")
                        sums = psC_p.tile([1, 2, QC], F32, tag="ts", name="sum")
                        exs = {}

                        # the two heads of a pair share kT/vnat, so their q
                        # columns ride one matmul stream (512 free): halves
                        # the PE instruction count and the ACT op count
                        def emit_scores(si):
                            delta = qc * QC - si * 128
                            psc = ps512_p.tile([128, 2, QC], F32, tag="sc", name="psc")
                            nc.tensor.matmul(
                                psc[:], kT[:, si * 128:(si + 1) * 128],
                                qT[:, hp:hp + 2, qsl],
                                start=True, stop=True,
                            )
                            ex = exp_p.tile([128, 2, QC], BF16, tag="ex")
                            nc.scalar.activation(ex[:], psc[:], AF.Exp)
                            for j in range(2):
                                if delta == 0:
                                    nc.vector.tensor_mul(
                                        ex[:, j, 0:128], ex[:, j, 0:128],
                                        mask_sb[:, 0, 128:256])
                                elif delta == -128:
                                    nc.vector.tensor_mul(
                                        ex[:, j, :], ex[:, j, :], mask_sb[:, 0, :])
                                elif delta == 896:
                                    nc.vector.tensor_mul(
                                        ex[:, j, 128:256], ex[:, j, 128:256],
                                        mask_sb[:, 1, 0:128])
                                elif delta == 1024:
                                    nc.vector.tensor_mul(
                                        ex[:, j, :], ex[:, j, :], mask_sb[:, 1, :])
                            exs[si] = ex

                        def emit_consume(si):
                            first = si == cons[0]
                            last = si == cons[-1]
                            ex = exs.pop(si)
                            nc.tensor.matmul(
                                sums[:], onc[:], ex[:],
                                start=first, stop=last,
                            )
                            nc.tensor.matmul(
                                avs[:], vnat[:, si, :], ex[:],
                                start=first, stop=last,
                            )

                        stag = min(4, len(sis))
                        for si in sis[:stag]:
                            emit_scores(si)
                        flush_norm2()
                        if hp == 0:
                            # previous slab's o_proj + RS, deferred here so
                            # the PE rides through the slab boundary on this
                            # slab's first score tiles
                            while oproj_q:
                                emit_oproj_group()
                        for idx, si in enumerate(sis[stag:]):
                            emit_scores(si)
                            emit_consume(cons[idx])
                        for k in range(stag, 0, -1):
                            emit_consume(cons[-k])

                        def make_norm(hp=hp, avs=avs, sums=sums, qsl=qsl):
                            def _norm():
                                sc_s = stB.tile([1, 2, QC], F32R, tag="rc", name="rc")
                                nc.scalar.activation(sc_s[:], sums[:], AF.Copy)
                                bcp = ps512_p.tile([128, 2, QC], F32, tag="sc", name="bcB")
                                nc.tensor.matmul(bcp[:], onr[:], sc_s[:], start=True, stop=True)
                                rcw = stB.tile([128, 2, QC], F32, tag="rcw", name="rcw")
                                nc.vector.reciprocal(rcw[:], bcp[:])
                                nc.vector.tensor_mul(
                                    attnT[:, hp:hp + 2, qsl], avs[:], rcw[:]
                                )
                            return _norm

                        pending_norm[0] = make_norm()
                    if qc >= 0:
                        flush_norm()
                        queue_oproj(qc)
                        while oproj_q:
                            emit_oproj_group()
                    else:
                        queue_oproj(qc)

    nc.finalize()
    return nc


_NC_CACHE = None


def _get_nc():
    global _NC_CACHE
    if _NC_CACHE is None:
        _NC_CACHE = _build()
    return _NC_CACHE


def _host_inputs(positions, hidden_states, w_qkv, q_norm_w, k_norm_w, w_o):
    positions = np.asarray(positions)
    hidden_states = np.asarray(hidden_states, dtype=np.float32)
    w_qkv = np.asarray(w_qkv, dtype=np.float32)
    q_norm_w = np.asarray(q_norm_w, dtype=np.float32)
    k_norm_w = np.asarray(k_norm_w, dtype=np.float32)
    w_o = np.asarray(w_o, dtype=np.float32)

    # [ki, tt, ko, j]
    hidT4 = np.ascontiguousarray(
        hidden_states.T.reshape(KO, 128, NT, 128).transpose(1, 2, 0, 3)
    ).astype(ml_dtypes.bfloat16)

    half = D // 2
    inv_freq = 1.0 / (THETA ** (np.arange(half, dtype=np.float32) / half))
    ang = positions.astype(np.float32)[:, None] * inv_freq[None, :]  # [T, 64]
    cos = np.cos(ang).T.astype(np.float32)   # [64, T]
    sin = np.sin(ang).T.astype(np.float32)
    cs2 = np.concatenate([cos, cos], axis=0)          # [128, T]
    sn2s = np.concatenate([-sin, sin], axis=0)        # [128, T]

    ss = np.arange(128)[:, None]
    ttv = np.arange(128)[None, :]
    tri_lo = (ttv >= ss).astype(np.float32)   # valid iff t >= s
    tri_hi = (ttv < ss).astype(np.float32)    # valid iff t < s
    zer = np.zeros_like(tri_lo)
    mk = np.ascontiguousarray(
        np.stack([
            np.concatenate([zer, tri_lo], axis=1),   # delta -128 full mask
            np.concatenate([tri_hi, zer], axis=1),   # delta 1024 full mask
        ], axis=1)
    ).astype(ml_dtypes.bfloat16)              # [128, 2, 256]

    qwv = (q_norm_w * (D ** -0.5)).reshape(D, 1).astype(np.float32)
    kwv = k_norm_w.reshape(D, 1).astype(np.float32)
    onc = np.ones((128, 1), ml_dtypes.bfloat16)
    onr = np.ones((1, 128), np.float32)

    in_maps = []
    for c in range(N_CORES):
        wq_c = np.concatenate(
            [
                w_qkv[:, c * HL * D:(c + 1) * HL * D],
                w_qkv[:, H * D + c * D:H * D + (c + 1) * D],
                w_qkv[:, (H + HKV) * D + c * D:(H + HKV) * D + (c + 1) * D],
            ],
            axis=1,
        )
        wq_c = np.ascontiguousarray(wq_c.reshape(KO, 128, (HL + 2) * D).transpose(1, 0, 2)).astype(ml_dtypes.bfloat16)
        wo_c = np.ascontiguousarray(
            w_o[c * HL * D:(c + 1) * HL * D, :].reshape(HL, 128, HID).transpose(1, 0, 2)
        ).astype(ml_dtypes.bfloat16)
        in_maps.append(
            {
                "hidT": hidT4,
                "wq": wq_c,
                "wo": wo_c,
                "cs2": cs2,
                "sn2s": sn2s,
                "masks": mk,
                "qwv": qwv,
                "kwv": kwv,
                "onc": onc,
                "onr": onr,
            }
        )
    return in_maps


def _assemble(results):
    out = np.empty((T, HID), np.float32)
    rows = QC // N_CORES
    half = rows // 2
    for c in range(N_CORES):
        r = np.asarray(results[c]["out"], dtype=np.float32)  # [N_QC, rows, HID]
        for qc in range(N_QC - 1):
            out[qc * QC + c * rows: qc * QC + (c + 1) * rows] = r[qc]
        # final slab was reduce-scattered as two 128-row halves
        qc = N_QC - 1
        base = qc * QC
        out[base + c * half: base + (c + 1) * half] = r[qc][:half]
        out[base + QC // 2 + c * half: base + QC // 2 + (c + 1) * half] = r[qc][half:]
    return out


def run_spmd(in_maps, trace=False, **kw):
    nc = _get_nc()
    return run_bass_kernel_spmd(nc, in_maps, list(range(N_CORES)), trace=trace, **kw)


def kernel(positions, hidden_states, w_qkv, q_norm_w, k_norm_w, w_o):
    in_maps = _host_inputs(positions, hidden_states, w_qkv, q_norm_w, k_norm_w, w_o)
    last_err = None
    for _ in range(3):
        try:
            res = run_spmd(in_maps)
            return _assemble(res.results)
        except Exception as e:  # rare transient NRT_EXEC_UNIT_UNRECOVERABLE
            last_err = e
    raise last_err
